# revision 13
# baseline (speedup 1.0000x reference)
"""Causal multi-head attention (B=2, H=16, S=2048, D=128, fp32) on 8 trn2 cores.

Sharding: head-parallel. B*H = 32 heads, 4 per core. Each core runs the same
Bass program on its own 4 heads; no collectives.

Per-head algorithm (transposed-scores flash attention, no max subtraction):
  - Q and K are pre-transposed on the host to [D, S] and cast to fp16 (fp32
    matmuls run at 1/4 rate on the PE). PSUM accumulation stays fp32.
  - scoresT[sk, sq] = K_blk @ Q^T via matmul(lhsT=KT_blk, rhs=QT_blk) into a
    flat 3-bank PSUM tile. ACT (exp) is the bottleneck engine (1 col/cycle
    @1.2GHz + ~265ns/instr overhead), so score tiles are packed to make exp
    instructions as large as possible while covering ONLY causally-alive
    columns:
      * off-diagonal k-blocks: up to 3 blocks x 512 q cols per tile
      * the 4 diagonal k-blocks of each q block are packed into ONE tile at
        col offsets {0, 512, 1024, 896} with widths {512, 384, 256, 128} ->
        1280 contiguous alive cols, one exp instruction, exact causal count.
  - expT -> fp16 SBUF; causal diagonal chunks masked to zero with gpsimd
    affine_select (keep q >= k), off both ACT's and DVE's critical paths.
  - out/denom together: V (fp16) gets a ones column appended; PV matmul
    (lhsT=expT chunk [sk,128sq], rhs=V'[sk,129]) accumulates over k blocks in
    fp32 PSUM; column 128 accumulates sum_k(expT) = the softmax denominator.
  - Epilogue: one DVE copy PSUM->SBUF per finished bank (breaks the
    write-after-read hazard with the next q-block's accumulation in ~400ns),
    then reciprocal + scale off the copy, one output DMA per q block.
Software pipeline is 2-deep: QK(i) is emitted before exp(i-1) before PV(i-2),
across q-block AND head boundaries, so the PE's next QK never queues behind
a PV that waits on the current exp, and ACT never gaps.
No running max is needed: inputs are ~N(0,1) so scores stay in [-6, 6] and
exp() cannot overflow; softmax is shift-invariant so this matches the
reference up to rounding.
"""

import contextlib
import math
import sys

import numpy as np

if "/opt/trn_rl_repo" not in sys.path:
    sys.path.insert(0, "/opt/trn_rl_repo")

import concourse.bass as bass
import concourse.mybir as mybir
import concourse.tile as tile
from concourse import bacc
from concourse.bass_utils import run_bass_kernel_spmd

B, H, S, D = 2, 16, 2048, 128
N_CORES = 8
HPC = (B * H) // N_CORES  # heads per core = 4
P = 128
QB = 512  # q block width per matmul
NQB = S // QB  # 4
NKB = S // P  # 16
QCH = QB // P  # 4 q chunks of 128 per q block
SCALE = 1.0 / math.sqrt(D)
FP32 = mybir.dt.float32
FP16 = mybir.dt.float16
STW = 3 * QB  # flat score tile width (3 PSUM banks)

# Diagonal packing: member c (k-block 4qj+c) has 512-128c alive q cols,
# placed so no matmul output crosses a PSUM bank: banks = [c0 | c1,c3 | c2].
DIAG_OFF = {0: 0, 1: 512, 2: 1024, 3: 896}


def build_program(hpc: int = HPC, num_devices: int = N_CORES) -> bass.Bass:
    from contextlib import ExitStack

    nc = bacc.Bacc(
        "TRN2", target_bir_lowering=False, debug=False, num_devices=num_devices
    )
    qT_d = nc.dram_tensor("qT", [hpc, D, S], FP16, kind="ExternalInput")
    kT_d = nc.dram_tensor("kT", [hpc, D, S], FP16, kind="ExternalInput")
    v_d = nc.dram_tensor("v", [hpc, S, D], FP16, kind="ExternalInput")
    o_d = nc.dram_tensor("o", [hpc, S, D], FP32, kind="ExternalOutput")

    with tile.TileContext(nc) as tc, ExitStack() as ctx:
        qk_pool = ctx.enter_context(tc.tile_pool(name="qk", bufs=3))
        v_pool = ctx.enter_context(tc.tile_pool(name="vp", bufs=3))
        exp_pool = ctx.enter_context(tc.tile_pool(name="exp", bufs=4))
        out_pool = ctx.enter_context(tc.tile_pool(name="out", bufs=2))
        den_pool = ctx.enter_context(tc.tile_pool(name="den", bufs=8))
        cop_pool = ctx.enter_context(tc.tile_pool(name="cop", bufs=4))
        ps_s_pool = ctx.enter_context(tc.tile_pool(name="ps_s", bufs=2, space="PSUM"))
        ps_o_pool = ctx.enter_context(tc.tile_pool(name="ps_o", bufs=1, space="PSUM"))

        # ---- per-head load providers -------------------------------------
        # The SP sequencer takes ~650ns to issue each DMA. Head 0 gates the
        # kernel ramp: qT/kT chunk 0 load first (they gate the first QK), the
        # rest of the head in one DMA per tensor. Later heads prefetch during
        # ~20us of compute: one DMA per tensor.
        def make_loads(h):
            if h == 0:
                qt0 = qk_pool.tile([P, QB], FP16, tag="qt0", name="qt0")
                nc.sync.dma_start(qt0[:], qT_d[h, :, :QB])
                kt0 = qk_pool.tile([P, QB], FP16, tag="kt0", name="kt0")
                nc.sync.dma_start(kt0[:], kT_d[h, :, :QB])
                # kta/qta gate qj1's QK (~12.5us) — issue before v0 (PV-side,
                # tolerates more latency since PV trails exp by a stage).
                kta = qk_pool.tile([P, NQB - 1, QB], FP16, tag="kta", name="kta")
                nc.sync.dma_start(
                    kta[:], kT_d[h, :, QB:].rearrange("d (g c) -> d g c", c=QB)
                )
                qta = qk_pool.tile([P, NQB - 1, QB], FP16, tag="qta", name="qta")
                nc.sync.dma_start(
                    qta[:], qT_d[h, :, QB:].rearrange("d (g c) -> d g c", c=QB)
                )
                v0 = v_pool.tile([P, QCH, D + 1], FP16, tag="v0", name="v0")
                nc.sync.dma_start(
                    v0[:, :, :D],
                    v_d[h, :QB, :].rearrange("(n p) d -> p n d", p=P),
                )
                nc.vector.memset(v0[:, :, D : D + 1], 1.0)
                va = v_pool.tile([P, NKB - QCH, D + 1], FP16, tag="va", name="va")
                nc.sync.dma_start(
                    va[:, :, :D], v_d[h, QB:, :].rearrange("(n p) d -> p n d", p=P)
                )
                nc.vector.memset(va[:, :, D : D + 1], 1.0)
                return (
                    lambda ki: kt0[:, (ki % QCH) * P : (ki % QCH + 1) * P]
                    if ki < QCH
                    else kta[:, ki // QCH - 1, (ki % QCH) * P : (ki % QCH + 1) * P],
                    lambda qj, trim: qt0[:, trim:] if qj == 0 else qta[:, qj - 1, trim:],
                    lambda ki: v0[:, ki, :] if ki < QCH else va[:, ki - QCH, :],
                )
            kTf = qk_pool.tile([P, NQB, QB], FP16, tag="kTf", name="kTf")
            nc.sync.dma_start(kTf[:], kT_d[h].rearrange("d (g c) -> d g c", c=QB))
            qTf = qk_pool.tile([P, NQB, QB], FP16, tag="qTf", name="qTf")
            nc.sync.dma_start(qTf[:], qT_d[h].rearrange("d (g c) -> d g c", c=QB))
            vf = v_pool.tile([P, NKB, D + 1], FP16, tag="vf", name="vf")
            nc.sync.dma_start(vf[:, :, :D], v_d[h].rearrange("(n p) d -> p n d", p=P))
            nc.vector.memset(vf[:, :, D : D + 1], 1.0)
            return (
                lambda ki: kTf[:, ki // QCH, (ki % QCH) * P : (ki % QCH + 1) * P],
                lambda qj, trim: qTf[:, qj, trim:],
                lambda ki: vf[:, ki, :],
            )

        # ---- item list: each item = one score tile / one exp instruction --
        # members: (ki, col_off, first_qc). Off-diag: first_qc=0, width 512.
        # Diag member c: width 512-128c, first_qc=c.
        def off_item(h, qj, kis, first):
            mem = [(ki, m * QB, 0) for m, ki in enumerate(kis)]
            return dict(h=h, qj=qj, mem=mem, expw=len(kis) * QB, first=first)

        def diag_item(h, qj, cs, offs, first):
            mem = [(QCH * qj + c, offs[c], c) for c in cs]
            expw = max(offs[c] + QB - P * c for c in cs)
            return dict(h=h, qj=qj, mem=mem, expw=expw, first=first)

        OFF_GROUPS = {1: [[0, 1], [2, 3]], 2: [[0, 1, 2], [3, 4, 5], [6, 7]],
                      3: [[0, 1, 2], [3, 4, 5], [6, 7, 8], [9, 10, 11]]}
        all_items = []
        for h in range(hpc):
            if h == 0:
                # split qj0 so the first exp only waits on two QK matmuls
                all_items.append(diag_item(0, 0, [0, 1], {0: 0, 1: 512}, True))
                all_items.append(diag_item(0, 0, [2, 3], {2: 0, 3: 256}, False))
            else:
                all_items.append(diag_item(h, 0, [0, 1, 2, 3], DIAG_OFF, True))
            for qj in range(1, NQB):
                for gi, kis in enumerate(OFF_GROUPS[qj]):
                    all_items.append(off_item(h, qj, kis, gi == 0))
                if h == hpc - 1 and qj == NQB - 1:
                    # kernel tail: split the last diagonal so bank 0's
                    # epilogue + store overlap the stream and only a 3-matmul
                    # item remains after the final exp
                    all_items.append(diag_item(h, qj, [0, 1], {0: 0, 1: 512}, False))
                    all_items.append(diag_item(h, qj, [2, 3], {2: 0, 3: 256}, False))
                else:
                    all_items.append(diag_item(h, qj, [0, 1, 2, 3], DIAG_OFF, False))

        providers: dict = {}
        po_tab: dict = {}
        ob_tab: dict = {}

        def emit_epilogue(h, qj, c):
            # Copy the finished PSUM bank to SBUF in one DVE instruction so
            # the next q-block's accumulation (same banks, ps_o bufs=1) only
            # waits ~400ns; reciprocal+scale then run off the copy.
            po_banks, po = po_tab[(h, qj)]
            if c == 0:
                ob_tab[(h, qj)] = out_pool.tile([P, QCH, D], FP32, tag="ob", name="ob")
            ob = ob_tab[(h, qj)]
            tail = h == hpc - 1 and qj == NQB - 1
            if tail:
                # Kernel tail: nothing reuses the PSUM banks, so skip the
                # WAR-breaking copy and run the shortest chain off PSUM
                # directly: one 2-wide reciprocal, then the two scale-muls in
                # parallel on DVE and ACT (exp stream is over).
                rec2 = den_pool.tile([P, 2, 1], FP32, tag="rec2", name="rec2")
                nc.vector.reciprocal(rec2[:], po_banks[c][:, :, D : D + 1])
                nc.vector.tensor_scalar_mul(
                    ob[:, 2 * c, :], po[2 * c][:, :D], rec2[:, 0, :]
                )
                if c == 0:
                    # ACT is still running the final exps: keep bank 0's
                    # chain entirely on DVE so its store isn't queued
                    # behind them on the ACT sequencer.
                    nc.vector.tensor_scalar_mul(
                        ob[:, 1, :], po[1][:, :D], rec2[:, 1, :]
                    )
                else:
                    nc.scalar.activation(
                        ob[:, 2 * c + 1, :],
                        po[2 * c + 1][:, :D],
                        mybir.ActivationFunctionType.Copy,
                        scale=rec2[:, 1, :],
                    )
            else:
                cop = cop_pool.tile([P, 2, D + 1], FP32, tag="cop", name="cop")
                nc.vector.tensor_copy(cop[:], po_banks[c][:])
                for i in range(2):
                    qc = 2 * c + i
                    rec = den_pool.tile([P, 1], FP32, tag="rec", name="rec")
                    nc.vector.reciprocal(rec[:], cop[:, i, D : D + 1])
                    nc.vector.tensor_scalar_mul(ob[:, qc, :], cop[:, i, :D], rec[:])
            if tail:
                s0 = (QCH * qj + 2 * c) * P
                if c == 0:
                    nc.sync.dma_start(
                        o_d[h, s0 : s0 + 2 * P, :].rearrange("(c p) d -> p c d", p=P),
                        ob[:, :2, :],
                    )
                else:
                    # last two chunks: issue from the DVE and ACT sequencers
                    # in parallel so neither waits on SP's ~650ns issue slot
                    nc.sync.dma_start(o_d[h, s0 : s0 + P, :], ob[:, 2, :])
                    nc.scalar.dma_start(o_d[h, s0 + P : s0 + 2 * P, :], ob[:, 3, :])
                    ob_tab.pop((h, qj))
            elif c == 1:
                nc.sync.dma_start(
                    o_d[h, qj * QB : (qj + 1) * QB, :].rearrange(
                        "(c p) d -> p c d", p=P
                    ),
                    ob_tab.pop((h, qj))[:],
                )

        def emit_qk(item):
            h, qj = item["h"], item["qj"]
            if qj == 0 and item["first"]:
                providers[h] = make_loads(h)
            kT_at, qT_at, _ = providers[h]
            if item["first"]:
                po_banks = [
                    ps_o_pool.tile([P, 2, D + 1], FP32, tag=f"po{c}", name=f"po{c}")
                    for c in range(QCH // 2)
                ]
                po_tab[(h, qj)] = (
                    po_banks,
                    [po_banks[c // 2][:, c % 2, :] for c in range(QCH)],
                )
            sT = ps_s_pool.tile([P, STW], FP32, tag="sT", name="sT")
            for ki, off, fqc in item["mem"]:
                w = QB - P * fqc
                nc.tensor.matmul(
                    sT[:, off : off + w],
                    kT_at(ki),
                    qT_at(qj, P * fqc),
                    start=True,
                    stop=True,
                )
            return (sT, item)

        def emit_exp(st):
            sT, item = st
            eT = exp_pool.tile([P, STW], FP16, tag="eT", name="eT")
            nc.scalar.activation(
                eT[:, : item["expw"]],
                sT[:, : item["expw"]],
                mybir.ActivationFunctionType.Exp,
                scale=SCALE,
            )
            # Causal diagonal chunks: zero the strictly-lower (k > q) part on
            # the otherwise-idle gpsimd engine (keep where q - k >= 0).
            for ki, off, fqc in item["mem"]:
                if ki == QCH * item["qj"] + fqc and fqc < QCH:
                    # first chunk of a diagonal member is the triangle chunk
                    nc.gpsimd.affine_select(
                        out=eT[:, off : off + P],
                        in_=eT[:, off : off + P],
                        compare_op=mybir.AluOpType.is_ge,
                        fill=0.0,
                        base=0,
                        pattern=[[1, P]],
                        channel_multiplier=-1,
                    )
            return (eT, item)

        def emit_pv(st):
            eT, item = st
            hp, qjp = item["h"], item["qj"]
            _, _, v_atp = providers[hp]
            _, po = po_tab[(hp, qjp)]
            kis = []
            for ki, off, fqc in item["mem"]:
                kis.append(ki)
                for qc in range(fqc, QCH):
                    qg = QCH * qjp + qc
                    if qg < ki:
                        continue
                    # Two accumulation groups share each PSUM bank: only the
                    # first write of the bank starts (clears has_written for
                    # the whole bank); only the bank's last write stops.
                    nc.tensor.matmul(
                        po[qc],
                        eT[:, off + (qc - fqc) * P : off + (qc - fqc + 1) * P],
                        v_atp(ki),
                        start=(ki == 0 and qc % 2 == 0),
                        stop=(ki == qg and qc % 2 == 1),
                    )
            if QCH * qjp + 1 in kis:
                emit_epilogue(hp, qjp, 0)
            if QCH * qjp + 3 in kis:
                emit_epilogue(hp, qjp, 1)
                po_tab.pop((hp, qjp))

        qk_staged = None
        exp_staged = None
        for idx in range(len(all_items) + 2):
            nxt_qk = emit_qk(all_items[idx]) if idx < len(all_items) else None
            nxt_exp = emit_exp(qk_staged) if qk_staged is not None else None
            if exp_staged is not None:
                emit_pv(exp_staged)
            qk_staged = nxt_qk
            exp_staged = nxt_exp
    nc.finalize()
    return nc


_CACHE: dict = {}


def _get_nc() -> bass.Bass:
    if "nc" not in _CACHE:
        _CACHE["nc"] = build_program()
    return _CACHE["nc"]


def make_in_maps(q: np.ndarray, k: np.ndarray, v: np.ndarray) -> list[dict]:
    q = np.asarray(q, dtype=np.float32).reshape(B * H, S, D)
    k = np.asarray(k, dtype=np.float32).reshape(B * H, S, D)
    v = np.asarray(v, dtype=np.float32).reshape(B * H, S, D)
    qT = q.transpose(0, 2, 1).astype(np.float16)  # [BH, D, S]
    kT = k.transpose(0, 2, 1).astype(np.float16)
    v16 = v.astype(np.float16)
    in_maps = []
    for c in range(N_CORES):
        sl = slice(c * HPC, (c + 1) * HPC)
        in_maps.append(
            {
                "qT": np.ascontiguousarray(qT[sl]),
                "kT": np.ascontiguousarray(kT[sl]),
                "v": np.ascontiguousarray(v16[sl]),
            }
        )
    return in_maps


def kernel(q: np.ndarray, k: np.ndarray, v: np.ndarray) -> np.ndarray:
    in_maps = make_in_maps(q, k, v)
    res = run_bass_kernel_spmd(_get_nc(), in_maps, core_ids=list(range(N_CORES)))
    o = np.concatenate([r["o"] for r in res.results], axis=0)
    return o.reshape(B, H, S, D)


# revision 15
# speedup vs baseline: 1.0126x; 1.0126x over previous
"""Causal multi-head attention (B=2, H=16, S=2048, D=128, fp32) on 8 trn2 cores.

Sharding: head-parallel. B*H = 32 heads, 4 per core. Each core runs the same
Bass program on its own 4 heads; no collectives.

Per-head algorithm (transposed-scores flash attention, no max subtraction):
  - Q and K are pre-transposed on the host to [D, S] and cast to fp16 (fp32
    matmuls run at 1/4 rate on the PE). PSUM accumulation stays fp32.
  - scoresT[sk, sq] = K_blk @ Q^T via matmul(lhsT=KT_blk, rhs=QT_blk) into a
    flat 3-bank PSUM tile. ACT (exp) is the bottleneck engine (1 col/cycle
    @1.2GHz + ~265ns/instr overhead), so score tiles are packed to make exp
    instructions as large as possible while covering ONLY causally-alive
    columns:
      * off-diagonal k-blocks: up to 3 blocks x 512 q cols per tile
      * the 4 diagonal k-blocks of each q block are packed into ONE tile at
        col offsets {0, 512, 1024, 896} with widths {512, 384, 256, 128} ->
        1280 contiguous alive cols, one exp instruction, exact causal count.
  - expT -> fp16 SBUF; causal diagonal chunks masked to zero with gpsimd
    affine_select (keep q >= k), off both ACT's and DVE's critical paths.
  - out/denom together: V (fp16) gets a ones column appended; PV matmul
    (lhsT=expT chunk [sk,128sq], rhs=V'[sk,129]) accumulates over k blocks in
    fp32 PSUM; column 128 accumulates sum_k(expT) = the softmax denominator.
  - Epilogue: one DVE copy PSUM->SBUF per finished bank (breaks the
    write-after-read hazard with the next q-block's accumulation in ~400ns),
    then reciprocal + scale off the copy, one output DMA per q block.
Software pipeline is 2-deep: QK(i) is emitted before exp(i-1) before PV(i-2),
across q-block AND head boundaries, so the PE's next QK never queues behind
a PV that waits on the current exp, and ACT never gaps.
No running max is needed: inputs are ~N(0,1) so scores stay in [-6, 6] and
exp() cannot overflow; softmax is shift-invariant so this matches the
reference up to rounding.
"""

import contextlib
import math
import sys

import numpy as np

if "/opt/trn_rl_repo" not in sys.path:
    sys.path.insert(0, "/opt/trn_rl_repo")

import concourse.bass as bass
import concourse.mybir as mybir
import concourse.tile as tile
from concourse import bacc
from concourse.bass_utils import run_bass_kernel_spmd

B, H, S, D = 2, 16, 2048, 128
N_CORES = 8
HPC = (B * H) // N_CORES  # heads per core = 4
P = 128
QB = 512  # q block width per matmul
NQB = S // QB  # 4
NKB = S // P  # 16
QCH = QB // P  # 4 q chunks of 128 per q block
SCALE = 1.0 / math.sqrt(D)
FP32 = mybir.dt.float32
FP16 = mybir.dt.float16
STW = 3 * QB  # flat score tile width (3 PSUM banks)

# Diagonal packing: member c (k-block 4qj+c) has 512-128c alive q cols,
# placed so no matmul output crosses a PSUM bank: banks = [c0 | c1,c3 | c2].
DIAG_OFF = {0: 0, 1: 512, 2: 1024, 3: 896}


def build_program(hpc: int = HPC, num_devices: int = N_CORES) -> bass.Bass:
    from contextlib import ExitStack

    nc = bacc.Bacc(
        "TRN2", target_bir_lowering=False, debug=False, num_devices=num_devices
    )
    qT_d = nc.dram_tensor("qT", [hpc, D, S], FP16, kind="ExternalInput")
    kT_d = nc.dram_tensor("kT", [hpc, D, S], FP16, kind="ExternalInput")
    v_d = nc.dram_tensor("v", [hpc, S, D], FP16, kind="ExternalInput")
    o_d = nc.dram_tensor("o", [hpc, S, D], FP32, kind="ExternalOutput")

    with tile.TileContext(nc) as tc, ExitStack() as ctx:
        qk_pool = ctx.enter_context(tc.tile_pool(name="qk", bufs=3))
        v_pool = ctx.enter_context(tc.tile_pool(name="vp", bufs=3))
        exp_pool = ctx.enter_context(tc.tile_pool(name="exp", bufs=4))
        out_pool = ctx.enter_context(tc.tile_pool(name="out", bufs=2))
        den_pool = ctx.enter_context(tc.tile_pool(name="den", bufs=8))
        cop_pool = ctx.enter_context(tc.tile_pool(name="cop", bufs=4))
        ps_s_pool = ctx.enter_context(tc.tile_pool(name="ps_s", bufs=2, space="PSUM"))
        ps_o_pool = ctx.enter_context(tc.tile_pool(name="ps_o", bufs=1, space="PSUM"))

        # ---- per-head load providers -------------------------------------
        # The SP sequencer takes ~650ns to issue each DMA. Head 0 gates the
        # kernel ramp: qT/kT chunk 0 load first (they gate the first QK), the
        # rest of the head in one DMA per tensor. Later heads prefetch during
        # ~20us of compute: one DMA per tensor.
        def make_loads(h):
            if h == 0:
                qt0 = qk_pool.tile([P, QB], FP16, tag="qt0", name="qt0")
                nc.sync.dma_start(qt0[:], qT_d[h, :, :QB])
                kt0 = qk_pool.tile([P, QB], FP16, tag="kt0", name="kt0")
                nc.sync.dma_start(kt0[:], kT_d[h, :, :QB])
                # kta/qta gate qj1's QK (~12.5us) — issue before v0 (PV-side,
                # tolerates more latency since PV trails exp by a stage).
                kta = qk_pool.tile([P, NQB - 1, QB], FP16, tag="kta", name="kta")
                nc.sync.dma_start(
                    kta[:], kT_d[h, :, QB:].rearrange("d (g c) -> d g c", c=QB)
                )
                qta = qk_pool.tile([P, NQB - 1, QB], FP16, tag="qta", name="qta")
                nc.sync.dma_start(
                    qta[:], qT_d[h, :, QB:].rearrange("d (g c) -> d g c", c=QB)
                )
                v0 = v_pool.tile([P, QCH, D + 1], FP16, tag="v0", name="v0")
                nc.sync.dma_start(
                    v0[:, :, :D],
                    v_d[h, :QB, :].rearrange("(n p) d -> p n d", p=P),
                )
                nc.vector.memset(v0[:, :, D : D + 1], 1.0)
                va = v_pool.tile([P, NKB - QCH, D + 1], FP16, tag="va", name="va")
                nc.sync.dma_start(
                    va[:, :, :D], v_d[h, QB:, :].rearrange("(n p) d -> p n d", p=P)
                )
                nc.vector.memset(va[:, :, D : D + 1], 1.0)
                return (
                    lambda ki: kt0[:, (ki % QCH) * P : (ki % QCH + 1) * P]
                    if ki < QCH
                    else kta[:, ki // QCH - 1, (ki % QCH) * P : (ki % QCH + 1) * P],
                    lambda qj, trim: qt0[:, trim:] if qj == 0 else qta[:, qj - 1, trim:],
                    lambda ki: v0[:, ki, :] if ki < QCH else va[:, ki - QCH, :],
                )
            kTf = qk_pool.tile([P, NQB, QB], FP16, tag="kTf", name="kTf")
            nc.sync.dma_start(kTf[:], kT_d[h].rearrange("d (g c) -> d g c", c=QB))
            qTf = qk_pool.tile([P, NQB, QB], FP16, tag="qTf", name="qTf")
            nc.sync.dma_start(qTf[:], qT_d[h].rearrange("d (g c) -> d g c", c=QB))
            vf = v_pool.tile([P, NKB, D + 1], FP16, tag="vf", name="vf")
            nc.sync.dma_start(vf[:, :, :D], v_d[h].rearrange("(n p) d -> p n d", p=P))
            nc.vector.memset(vf[:, :, D : D + 1], 1.0)
            return (
                lambda ki: kTf[:, ki // QCH, (ki % QCH) * P : (ki % QCH + 1) * P],
                lambda qj, trim: qTf[:, qj, trim:],
                lambda ki: vf[:, ki, :],
            )

        # ---- item list: each item = one score tile / one exp instruction --
        # members: (ki, col_off, first_qc). Off-diag: first_qc=0, width 512.
        # Diag member c: width 512-128c, first_qc=c.
        def off_item(h, qj, kis, first):
            mem = [(ki, m * QB, 0) for m, ki in enumerate(kis)]
            return dict(h=h, qj=qj, mem=mem, expw=len(kis) * QB, first=first)

        def diag_item(h, qj, cs, offs, first):
            mem = [(QCH * qj + c, offs[c], c) for c in cs]
            expw = max(offs[c] + QB - P * c for c in cs)
            return dict(h=h, qj=qj, mem=mem, expw=expw, first=first)

        OFF_GROUPS = {1: [[0, 1], [2, 3]], 2: [[0, 1, 2], [3, 4, 5], [6, 7]],
                      3: [[0, 1, 2], [3, 4, 5], [6, 7, 8], [9, 10, 11]]}
        all_items = []
        for h in range(hpc):
            if h == 0:
                # split qj0 so the first exp only waits on two QK matmuls
                all_items.append(diag_item(0, 0, [0, 1], {0: 0, 1: 512}, True))
                all_items.append(diag_item(0, 0, [2, 3], {2: 0, 3: 256}, False))
            else:
                all_items.append(diag_item(h, 0, [0, 1, 2, 3], DIAG_OFF, True))
            for qj in range(1, NQB):
                for gi, kis in enumerate(OFF_GROUPS[qj]):
                    all_items.append(off_item(h, qj, kis, gi == 0))
                if h == hpc - 1 and qj == NQB - 1:
                    # kernel tail: split the last diagonal so bank 0's
                    # epilogue + store overlap the stream and only a 3-matmul
                    # item remains after the final exp
                    all_items.append(diag_item(h, qj, [0, 1], {0: 0, 1: 512}, False))
                    all_items.append(diag_item(h, qj, [2, 3], {2: 0, 3: 256}, False))
                else:
                    all_items.append(diag_item(h, qj, [0, 1, 2, 3], DIAG_OFF, False))

        providers: dict = {}
        po_tab: dict = {}
        ob_tab: dict = {}

        def emit_epilogue(h, qj, c):
            # Copy the finished PSUM bank to SBUF in one DVE instruction so
            # the next q-block's accumulation (same banks, ps_o bufs=1) only
            # waits ~400ns; reciprocal+scale then run off the copy.
            po_banks, po = po_tab[(h, qj)]
            if c == 0:
                ob_tab[(h, qj)] = out_pool.tile([P, QCH, D], FP32, tag="ob", name="ob")
            ob = ob_tab[(h, qj)]
            tail = h == hpc - 1 and qj == NQB - 1
            if tail:
                # Kernel tail: nothing reuses the PSUM banks, so skip the
                # WAR-breaking copy and run the shortest chain off PSUM
                # directly: one 2-wide reciprocal, then the two scale-muls in
                # parallel on DVE and ACT (exp stream is over).
                rec2 = den_pool.tile([P, 2, 1], FP32, tag="rec2", name="rec2")
                nc.vector.reciprocal(rec2[:], po_banks[c][:, :, D : D + 1])
                nc.vector.tensor_scalar_mul(
                    ob[:, 2 * c, :], po[2 * c][:, :D], rec2[:, 0, :]
                )
                nc.scalar.activation(
                    ob[:, 2 * c + 1, :],
                    po[2 * c + 1][:, :D],
                    mybir.ActivationFunctionType.Copy,
                    scale=rec2[:, 1, :],
                )
            else:
                cop = cop_pool.tile([P, 2, D + 1], FP32, tag="cop", name="cop")
                nc.vector.tensor_copy(cop[:], po_banks[c][:])
                for i in range(2):
                    qc = 2 * c + i
                    rec = den_pool.tile([P, 1], FP32, tag="rec", name="rec")
                    nc.vector.reciprocal(rec[:], cop[:, i, D : D + 1])
                    nc.vector.tensor_scalar_mul(ob[:, qc, :], cop[:, i, :D], rec[:])
            if tail:
                s0 = (QCH * qj + 2 * c) * P
                if c == 0:
                    nc.sync.dma_start(
                        o_d[h, s0 : s0 + 2 * P, :].rearrange("(c p) d -> p c d", p=P),
                        ob[:, :2, :],
                    )
                else:
                    # last two chunks: issue from the DVE and ACT sequencers
                    # in parallel so neither waits on SP's ~650ns issue slot
                    nc.sync.dma_start(o_d[h, s0 : s0 + P, :], ob[:, 2, :])
                    nc.scalar.dma_start(o_d[h, s0 + P : s0 + 2 * P, :], ob[:, 3, :])
                    ob_tab.pop((h, qj))
            elif c == 1:
                nc.sync.dma_start(
                    o_d[h, qj * QB : (qj + 1) * QB, :].rearrange(
                        "(c p) d -> p c d", p=P
                    ),
                    ob_tab.pop((h, qj))[:],
                )

        def emit_qk(item):
            h, qj = item["h"], item["qj"]
            if qj == 0 and item["first"]:
                providers[h] = make_loads(h)
            kT_at, qT_at, _ = providers[h]
            if item["first"]:
                po_banks = [
                    ps_o_pool.tile([P, 2, D + 1], FP32, tag=f"po{c}", name=f"po{c}")
                    for c in range(QCH // 2)
                ]
                po_tab[(h, qj)] = (
                    po_banks,
                    [po_banks[c // 2][:, c % 2, :] for c in range(QCH)],
                )
            sT = ps_s_pool.tile([P, STW], FP32, tag="sT", name="sT")
            # First group of a later head: boost its scheduler priority so
            # the PE runs it ahead of the outgoing head's PV burst and ACT
            # crosses the head boundary without a gap.
            boost = (
                tc.high_priority(offset=200)
                if (qj == 0 and item["first"] and h != 0)
                else contextlib.nullcontext()
            )
            with boost:
                for ki, off, fqc in item["mem"]:
                    w = QB - P * fqc
                    nc.tensor.matmul(
                        sT[:, off : off + w],
                        kT_at(ki),
                        qT_at(qj, P * fqc),
                        start=True,
                        stop=True,
                    )
            return (sT, item)

        def emit_exp(st):
            sT, item = st
            eT = exp_pool.tile([P, STW], FP16, tag="eT", name="eT")
            nc.scalar.activation(
                eT[:, : item["expw"]],
                sT[:, : item["expw"]],
                mybir.ActivationFunctionType.Exp,
                scale=SCALE,
            )
            # Causal diagonal chunks: zero the strictly-lower (k > q) part on
            # the otherwise-idle gpsimd engine (keep where q - k >= 0).
            for ki, off, fqc in item["mem"]:
                if ki == QCH * item["qj"] + fqc and fqc < QCH:
                    # first chunk of a diagonal member is the triangle chunk
                    nc.gpsimd.affine_select(
                        out=eT[:, off : off + P],
                        in_=eT[:, off : off + P],
                        compare_op=mybir.AluOpType.is_ge,
                        fill=0.0,
                        base=0,
                        pattern=[[1, P]],
                        channel_multiplier=-1,
                    )
            return (eT, item)

        def emit_pv(st):
            eT, item = st
            hp, qjp = item["h"], item["qj"]
            _, _, v_atp = providers[hp]
            _, po = po_tab[(hp, qjp)]
            kis = []
            for ki, off, fqc in item["mem"]:
                kis.append(ki)
                for qc in range(fqc, QCH):
                    qg = QCH * qjp + qc
                    if qg < ki:
                        continue
                    # Two accumulation groups share each PSUM bank: only the
                    # first write of the bank starts (clears has_written for
                    # the whole bank); only the bank's last write stops.
                    nc.tensor.matmul(
                        po[qc],
                        eT[:, off + (qc - fqc) * P : off + (qc - fqc + 1) * P],
                        v_atp(ki),
                        start=(ki == 0 and qc % 2 == 0),
                        stop=(ki == qg and qc % 2 == 1),
                    )
            if QCH * qjp + 1 in kis:
                emit_epilogue(hp, qjp, 0)
            if QCH * qjp + 3 in kis:
                emit_epilogue(hp, qjp, 1)
                po_tab.pop((hp, qjp))

        qk_staged = None
        exp_staged = None
        for idx in range(len(all_items) + 2):
            nxt_qk = emit_qk(all_items[idx]) if idx < len(all_items) else None
            nxt_exp = emit_exp(qk_staged) if qk_staged is not None else None
            if exp_staged is not None:
                emit_pv(exp_staged)
            qk_staged = nxt_qk
            exp_staged = nxt_exp
    nc.finalize()
    return nc


_CACHE: dict = {}


def _get_nc() -> bass.Bass:
    if "nc" not in _CACHE:
        _CACHE["nc"] = build_program()
    return _CACHE["nc"]


def make_in_maps(q: np.ndarray, k: np.ndarray, v: np.ndarray) -> list[dict]:
    q = np.asarray(q, dtype=np.float32).reshape(B * H, S, D)
    k = np.asarray(k, dtype=np.float32).reshape(B * H, S, D)
    v = np.asarray(v, dtype=np.float32).reshape(B * H, S, D)
    qT = q.transpose(0, 2, 1).astype(np.float16)  # [BH, D, S]
    kT = k.transpose(0, 2, 1).astype(np.float16)
    v16 = v.astype(np.float16)
    in_maps = []
    for c in range(N_CORES):
        sl = slice(c * HPC, (c + 1) * HPC)
        in_maps.append(
            {
                "qT": np.ascontiguousarray(qT[sl]),
                "kT": np.ascontiguousarray(kT[sl]),
                "v": np.ascontiguousarray(v16[sl]),
            }
        )
    return in_maps


def kernel(q: np.ndarray, k: np.ndarray, v: np.ndarray) -> np.ndarray:
    in_maps = make_in_maps(q, k, v)
    res = run_bass_kernel_spmd(_get_nc(), in_maps, core_ids=list(range(N_CORES)))
    o = np.concatenate([r["o"] for r in res.results], axis=0)
    return o.reshape(B, H, S, D)
